# revision 1
# baseline (speedup 1.0000x reference)
"""GroupedQueryAttention Bass kernel for 8 Trainium2 NeuronCores.

Sharding: 8 devices = 2 batches x 4 sequence-quarters.
Device d handles batch b=d//4, query rows [512*i, 512*(i+1)) with i=d%4.

All matmuls run in fp16 (fp32 PSUM accumulation).  Uniform 16-bit dtype
keeps every matmul on the PE's 1-cycle/column path (mixing f32r with
16-bit demotes f32r matmuls to 2 cycles/column) and halves all HBM and
collective traffic; fp16's 10 mantissa bits keep quantization noise ~8x
below bf16.  A constant -10 shift folded into the exp bias (mathematically
a softmax no-op) keeps the unnormalized exp values inside fp16 range
(max logit+sink measured ~19.7; weakest row's max ~1.8, so all rows'
dominant weights stay in fp16 normal range).

Per device:
  - K/V projection for the local 512-row slice (+RoPE on K, V transposed
    to s-major), packed into one [1024, 512] fp16 slice; a single
    AllGather distributes it across the 4 devices of the batch.
  - Q projection (16 heads) overlaps the collective; all weights are
    resident in SBUF early so the gather's DMA traffic cannot starve it.
  - Attention: scores^T = matmul(lhsT=k^T, rhs=q^T); exp on ScalarE
    (scale + shifted sink bias) -> P^T fp16; out^T += matmul(lhsT=v,
    rhs=P^T); denominators via matmul(lhsT=ones).  Normalization: DVE reciprocal_approx_fast on the
    [1,512] sums + gpsimd partition broadcast + one DVE multiply.
  - o_proj uses out^T directly as lhsT with streamed fp16 Wo.

DMAs are batched into ~50 large transfers (a per-chunk scheme serializes
~500 dma_start issues on the sync sequencer and starves the PE).
"""

from contextlib import ExitStack

import numpy as np

import concourse.bass as bass
import concourse.tile as tile
from concourse import bacc, mybir
from concourse.bass_utils import run_bass_kernel_spmd
from concourse.masks import make_identity

F32 = mybir.dt.float32
FP16 = mybir.dt.float16
AF = mybir.ActivationFunctionType
ALU = mybir.AluOpType

# Problem dims (hardcoded per contract)
B = 2
S = 2048
E = 2048
HQ = 16
HKV = 4
D = 128
REP = HQ // HKV          # 4 q-heads per kv head
NDEV = 8
DPB = 4                  # devices per batch
SQ = S // DPB            # 512 local query rows
EC = E // 128            # 16 contraction chunks
SKC = S // 128           # 16 key chunks
SCALE = 1.0 / float(np.sqrt(D))
ESHIFT = 10.0            # exp(l - ESHIFT): softmax-invariant, keeps P in fp16 range

_CACHE = {}


def _build(sinks, with_bias_qkv, with_bias_o):
    nc = bacc.Bacc("TRN2", target_bir_lowering=False, debug=False, num_devices=NDEV)

    xT = nc.dram_tensor("xT", [E, SQ], FP16, kind="ExternalInput").ap()
    wq = nc.dram_tensor("wq", [E, HQ * D], FP16, kind="ExternalInput").ap()
    wk = nc.dram_tensor("wk", [E, HKV * D], FP16, kind="ExternalInput").ap()
    wv = nc.dram_tensor("wv", [E, HKV * D], FP16, kind="ExternalInput").ap()
    wo = nc.dram_tensor("wo", [HQ * D, E], FP16, kind="ExternalInput").ap()
    cosT = nc.dram_tensor("cosT", [D // 2, SQ], F32, kind="ExternalInput").ap()
    sinT = nc.dram_tensor("sinT", [D // 2, SQ], F32, kind="ExternalInput").ap()
    if with_bias_qkv:
        # laid out [D, H] so a column is the per-partition bias of one head
        bqd = nc.dram_tensor("bqd", [D, HQ], F32, kind="ExternalInput").ap()
        bkd = nc.dram_tensor("bkd", [D, HKV], F32, kind="ExternalInput").ap()
        bvd = nc.dram_tensor("bvd", [D, HKV], F32, kind="ExternalInput").ap()
    if with_bias_o:
        bod = nc.dram_tensor("bod", [1, E], F32, kind="ExternalInput").ap()
    out = nc.dram_tensor("out", [SQ, E], F32, kind="ExternalOutput").ap()

    with tile.TileContext(nc) as tc, ExitStack() as es:
        _emit(tc, es, locals(), sinks, with_bias_qkv, with_bias_o)
    nc.compile()
    return nc


def _emit(tc, es, t, sinks, with_bias_qkv, with_bias_o):
    nc = tc.nc
    xT, wq, wk, wv, wo = t["xT"], t["wq"], t["wk"], t["wv"], t["wo"]
    cosT, sinT, out = t["cosT"], t["sinT"], t["out"]

    # ---------- persistent pools ----------
    const_pool = es.enter_context(tc.tile_pool(name="const", bufs=1))
    dram = es.enter_context(tc.tile_pool(name="dram", bufs=1, space="DRAM"))

    ident_f = const_pool.tile([128, 128], F32, tag="ident_f")
    make_identity(nc, ident_f[:])
    ident = const_pool.tile([128, 128], FP16, tag="ident")
    nc.vector.tensor_copy(ident[:], ident_f[:])
    ones_h = const_pool.tile([128, 1], FP16, tag="ones_h")
    nc.vector.memset(ones_h[:], 1.0)

    if with_bias_qkv:
        bq_sb = const_pool.tile([D, HQ], F32, tag="bq")
        nc.sync.dma_start(bq_sb[:], t["bqd"])
        bk_sb = const_pool.tile([D, HKV], F32, tag="bk")
        nc.sync.dma_start(bk_sb[:], t["bkd"])
        bv_sb = const_pool.tile([D, HKV], F32, tag="bv")
        nc.sync.dma_start(bv_sb[:], t["bvd"])

    # exp bias: per-head sink minus the range shift
    sinks_sb = const_pool.tile([128, HQ], F32, tag="sinks")
    for _h in range(HQ):
        nc.vector.memset(sinks_sb[:, _h : _h + 1], float(sinks[_h]) - ESHIFT)

    # packed collective payload: rows 0-511 = k^T [h*128+d, s];
    # rows 512-1023 = v s-major [s, h*128+d]
    kv_slice = dram.tile([2 * HKV * D, SQ], FP16, tag="kvs")
    kv_gath = dram.tile([DPB, 2 * HKV * D, SQ], FP16, tag="kvg")

    def rope(dst, src_ps, n_heads, cos_t, sin_t, tmp_pool, bias_sb=None, head0=0):
        """dst/src: [128, n_heads*SQ]; halves along partitions. bias optional."""
        w = n_heads * SQ
        src = src_ps[:].rearrange("p (h s) -> p h s", h=n_heads)
        if bias_sb is not None:
            for j in range(n_heads):
                nc.vector.tensor_scalar_add(
                    src_ps[:, j * SQ : (j + 1) * SQ],
                    src_ps[:, j * SQ : (j + 1) * SQ],
                    bias_sb[:, head0 + j : head0 + j + 1],
                )
        dstv = dst[:].rearrange("p (h s) -> p h s", h=n_heads)
        cosb = cos_t[:, None, :].to_broadcast((64, n_heads, SQ))
        sinb = sin_t[:, None, :].to_broadcast((64, n_heads, SQ))
        q1 = src[0:64]
        q2 = src[64:128]
        m1 = tmp_pool.tile([64, w], F32, tag="m", name="m1")[:].rearrange("p (h s) -> p h s", h=n_heads)
        m2 = tmp_pool.tile([64, w], F32, tag="m", name="m2")[:].rearrange("p (h s) -> p h s", h=n_heads)
        nc.vector.tensor_tensor(m1, q1, cosb, ALU.mult)
        nc.vector.tensor_tensor(m2, q2, sinb, ALU.mult)
        nc.vector.tensor_tensor(dstv[0:64], m1, m2, ALU.subtract)
        m3 = tmp_pool.tile([64, w], F32, tag="m", name="m3")[:].rearrange("p (h s) -> p h s", h=n_heads)
        m4 = tmp_pool.tile([64, w], F32, tag="m", name="m4")[:].rearrange("p (h s) -> p h s", h=n_heads)
        nc.vector.tensor_tensor(m3, q2, cosb, ALU.mult)
        nc.vector.tensor_tensor(m4, q1, sinb, ALU.mult)
        nc.vector.tensor_tensor(dstv[64:128], m3, m4, ALU.add)

    # q_sb persists into attention
    q_sb = const_pool.tile([128, HQ * SQ], FP16, tag="q_sb")

    # ---------- phase 1+2: projections + rope + transpose + gather --------
    with (
        tc.tile_pool(name="p12", bufs=1) as p12,
        tc.tile_pool(name="proj_ps", bufs=3, space="PSUM") as proj_ps,
        tc.tile_pool(name="rope_tmp", bufs=4) as rope_tmp,
        tc.tile_pool(name="tr_ps", bufs=2, space="PSUM") as tr_ps,
        tc.tile_pool(name="vtr", bufs=1) as vtr,
    ):
        xT_sb = p12.tile([128, EC * SQ], FP16, tag="xT")
        xview = xT_sb[:].rearrange("p (c s) -> p c s", s=SQ)
        xdram = xT.rearrange("(c p) s -> p c s", p=128)
        for i in range(4):
            nc.sync.dma_start(xview[:, 4 * i : 4 * (i + 1), :], xdram[:, 4 * i : 4 * (i + 1), :])
        cos_sb = p12.tile([64, SQ], F32, tag="cos")
        nc.sync.dma_start(cos_sb[:], cosT)
        sin_sb = p12.tile([64, SQ], F32, tag="sin")
        nc.sync.dma_start(sin_sb[:], sinT)

        # full Wk / Wv / Wq resident in SBUF (fp16)
        wk_sb = p12.tile([128, HKV * EC * 128], FP16, tag="wk_sb")
        wv_sb = p12.tile([128, HKV * EC * 128], FP16, tag="wv_sb")
        wkv_view = {}
        for which, w_dram, sb in (("k", wk, wk_sb), ("v", wv, wv_sb)):
            view = sb[:].rearrange("p (h c n) -> p h c n", h=HKV, n=128)
            wd = w_dram.rearrange("(c p) (h n) -> p h c n", p=128, n=128)
            for h in range(HKV):
                nc.sync.dma_start(view[:, h], wd[:, h])
            wkv_view[which] = view
        k_sb = p12.tile([128, HKV * SQ], FP16, tag="k_sb")
        v_sb = p12.tile([128, HKV * SQ], FP16, tag="v_sb")
        vtr_sb = vtr.tile([128, (SQ // 128) * HKV * 128], FP16, tag="vts")
        vtr_view = vtr_sb[:].rearrange("p (sc h d) -> p sc h d", h=HKV, d=128)

        for which, sb in (("k", k_sb), ("v", v_sb)):
            for g in range(HKV // 2):   # 2 heads per psum group
                ps = proj_ps.tile([128, 2 * SQ], F32, tag="proj")
                for j in range(2):
                    h = g * 2 + j
                    for c in range(EC):
                        nc.tensor.matmul(
                            ps[:, j * SQ : (j + 1) * SQ],
                            wkv_view[which][:, h, c, :],
                            xview[:, c, :],
                            start=(c == 0),
                            stop=(c == EC - 1),
                        )
                if which == "k":
                    rope(
                        sb[:, g * 2 * SQ : (g + 1) * 2 * SQ],
                        ps, 2, cos_sb, sin_sb, rope_tmp,
                        bias_sb=(bk_sb if with_bias_qkv else None), head0=g * 2,
                    )
                    # stream this group's k rows out as soon as roped
                    nc.sync.dma_start(
                        kv_slice[g * 256 : (g + 1) * 256, :].rearrange(
                            "(h p) s -> p h s", p=128
                        ),
                        sb[:, g * 2 * SQ : (g + 1) * 2 * SQ].rearrange(
                            "p (h s) -> p h s", h=2
                        ),
                    )
                else:
                    if with_bias_qkv:
                        for j in range(2):
                            nc.vector.tensor_scalar_add(
                                ps[:, j * SQ : (j + 1) * SQ],
                                ps[:, j * SQ : (j + 1) * SQ],
                                bv_sb[:, g * 2 + j : g * 2 + j + 1],
                            )
                    nc.vector.tensor_copy(sb[:, g * 2 * SQ : (g + 1) * 2 * SQ], ps[:])
                    # transpose this group's v tiles right away and stream out
                    for j in range(2):
                        h = g * 2 + j
                        for sc in range(SQ // 128):
                            tp = tr_ps.tile([128, 128], FP16, tag="trp")
                            nc.tensor.transpose(
                                tp[:],
                                sb[:, h * SQ + sc * 128 : h * SQ + (sc + 1) * 128],
                                ident[:],
                            )
                            nc.vector.tensor_copy(vtr_view[:, sc, h, :], tp[:])
                    nc.sync.dma_start(
                        kv_slice[HKV * D :, g * 256 : (g + 1) * 256].rearrange(
                            "(sc p) hd -> p sc hd", p=128
                        ),
                        vtr_view[:, :, g * 2 : (g + 1) * 2, :].rearrange(
                            "p sc h d -> p sc (h d)"
                        ),
                    )

        nc.gpsimd.collective_compute(
            "AllGather",
            ALU.bypass,
            ins=[kv_slice[:].opt()],
            outs=[kv_gath[:].opt()],
            replica_groups=[[0, 1, 2, 3], [4, 5, 6, 7]],
        )

        # ---------- Q projection + rope (overlaps collective) ----
        wq_sb = p12.tile([128, HQ * EC * 128], FP16, tag="wq_sb")
        wq_view = wq_sb[:].rearrange("p (h c n) -> p h c n", h=HQ, n=128)
        wqd = wq.rearrange("(c p) (h n) -> p h c n", p=128, n=128)
        for h in range(HQ):
            nc.scalar.dma_start(wq_view[:, h], wqd[:, h])

        for g in range(HQ // 2):
            ps = proj_ps.tile([128, 2 * SQ], F32, tag="proj")
            for j in range(2):
                h = g * 2 + j
                for c in range(EC):
                    nc.tensor.matmul(
                        ps[:, j * SQ : (j + 1) * SQ],
                        wq_view[:, h, c, :],
                        xview[:, c, :],
                        start=(c == 0),
                        stop=(c == EC - 1),
                    )
            rope(
                q_sb[:, g * 2 * SQ : (g + 1) * 2 * SQ],
                ps, 2, cos_sb, sin_sb, rope_tmp,
                bias_sb=(bq_sb if with_bias_qkv else None), head0=g * 2,
            )

    # ---------- phase 3: attention ----------
    attn_sb = const_pool.tile([128, HQ * SQ], FP16, tag="attn_sb")  # out^T per head

    with (
        tc.tile_pool(name="kv_all", bufs=1) as kv_all,
        tc.tile_pool(name="wo_pool", bufs=2) as wo_pool,
        ExitStack() as attn_es,
    ):
        sc_ps = attn_es.enter_context(tc.tile_pool(name="sc_ps", bufs=5, space="PSUM"))
        out_ps = attn_es.enter_context(tc.tile_pool(name="out_ps", bufs=2, space="PSUM"))
        sum_ps = attn_es.enter_context(tc.tile_pool(name="sum_ps", bufs=1, space="PSUM"))
        p_pool = attn_es.enter_context(tc.tile_pool(name="p_pool", bufs=6))
        den_pool = attn_es.enter_context(tc.tile_pool(name="den_pool", bufs=3))

        # full-sequence K^T and V (fp16) per kv head
        k_all = kv_all.tile([128, HKV * S], FP16, tag="k_all")   # [d, h*S + sk]
        v_all = kv_all.tile([128, HKV * S], FP16, tag="v_all")   # [s%128, h*S + c*128 + d]
        v_view = v_all[:].rearrange("p (h si sc d) -> p h si sc d", h=HKV, si=DPB, d=128)
        for h in range(HKV):
            nc.sync.dma_start(
                k_all[:, h * S : (h + 1) * S].rearrange("p (si s) -> p si s", si=DPB),
                kv_gath[:, h * 128 : (h + 1) * 128, :].rearrange("si p s -> p si s"),
            )
            for si in range(DPB):
                nc.sync.dma_start(
                    v_view[:, h, si],
                    kv_gath[si, HKV * D :, h * 128 : (h + 1) * 128].rearrange(
                        "(sc p) d -> p sc d", p=128
                    ),
                )

        for h in range(HQ):
            kh = h // REP
            op = out_ps.tile([128, SQ], F32, tag="outp")
            sp = sum_ps.tile([1, SQ], F32, tag="sump")
            for c in range(SKC):
                scp = sc_ps.tile([128, SQ], F32, tag="scp")
                nc.tensor.matmul(
                    scp[:],
                    k_all[:, kh * S + c * 128 : kh * S + (c + 1) * 128],
                    q_sb[:, h * SQ : (h + 1) * SQ],
                    start=True,
                    stop=True,
                )
                pt = p_pool.tile([128, SQ], FP16, tag="pt")
                nc.scalar.activation(pt[:], scp[:], AF.Exp, bias=sinks_sb[:, h : h + 1], scale=SCALE)
                nc.tensor.matmul(
                    op[:],
                    v_all[:, kh * S + c * 128 : kh * S + (c + 1) * 128],
                    pt[:],
                    start=(c == 0),
                    stop=(c == SKC - 1),
                    skip_group_check=True,
                )
                nc.tensor.matmul(
                    sp[:],
                    ones_h[:],
                    pt[:],
                    start=(c == 0),
                    stop=(c == SKC - 1),
                    skip_group_check=True,
                )
            rs = den_pool.tile([1, SQ], F32, tag="rs")
            nc.vector.reciprocal_approx_fast(rs[:], sp[:])
            den = den_pool.tile([128, SQ], F32, tag="den")
            nc.gpsimd.partition_broadcast(den[:], rs[:])
            nc.vector.tensor_tensor(
                attn_sb[:, h * SQ : (h + 1) * SQ], op[:], den[:], ALU.mult
            )

        # ---------- phase 4: o_proj ----------
        attn_es.close()
        with (
            tc.tile_pool(name="o_ps", bufs=2, space="PSUM") as o_ps,
            tc.tile_pool(name="o_sb", bufs=3) as o_sb_pool,
        ):
            if with_bias_o:
                bo_sb = const_pool.tile([1, E], F32, tag="bo")
                nc.sync.dma_start(bo_sb[:], t["bod"])
                bo_b = const_pool.tile([128, E], F32, tag="bo_b")
                nc.gpsimd.partition_broadcast(bo_b[:], bo_sb[:])
            wod = wo.rearrange("(c p) e -> p c e", p=128)
            outd = out.rearrange("(sq p) e -> p sq e", p=128)
            for et in range(4):
                wt = wo_pool.tile([128, EC * 512], FP16, tag="wo_et")
                wtv = wt[:].rearrange("p (c n) -> p c n", n=512)
                nc.scalar.dma_start(wtv, wod[:, :, et * 512 : (et + 1) * 512])
                for sqc in range(SQ // 128):
                    ps = o_ps.tile([128, 512], F32, tag="ops")
                    for hd in range(HQ):
                        nc.tensor.matmul(
                            ps[:],
                            attn_sb[:, hd * SQ + sqc * 128 : hd * SQ + (sqc + 1) * 128],
                            wtv[:, hd, :],
                            start=(hd == 0),
                            stop=(hd == HQ - 1),
                        )
                    ot = o_sb_pool.tile([128, 512], F32, tag="osb")
                    if with_bias_o:
                        nc.vector.tensor_tensor(
                            ot[:], ps[:], bo_b[:, et * 512 : (et + 1) * 512], ALU.add
                        )
                    else:
                        nc.scalar.copy(ot[:], ps[:])
                    nc.sync.dma_start(
                        outd[:, sqc, et * 512 : (et + 1) * 512], ot[:]
                    )


RUN_KWARGS = {}


def kernel(x, sin, cos, Wq, bq, Wk, bk, Wv, bv, Wo, bo, sinks):
    x = np.asarray(x, dtype=np.float32)
    sin = np.asarray(sin, dtype=np.float32)
    cos = np.asarray(cos, dtype=np.float32)
    sinks = np.asarray(sinks, dtype=np.float32)
    with_bias_qkv = bool(np.any(bq) or np.any(bk) or np.any(bv))
    with_bias_o = bool(np.any(bo))

    key = (sinks.tobytes(), with_bias_qkv, with_bias_o)
    if key not in _CACHE:
        _CACHE[key] = _build(sinks, with_bias_qkv, with_bias_o)
    nc = _CACHE[key]

    wq_h = np.ascontiguousarray(np.asarray(Wq, np.float32).astype(np.float16))
    wk_h = np.ascontiguousarray(np.asarray(Wk, np.float32).astype(np.float16))
    wv_h = np.ascontiguousarray(np.asarray(Wv, np.float32).astype(np.float16))
    wo_h = np.ascontiguousarray(np.asarray(Wo, np.float32).astype(np.float16))

    in_maps = []
    for dev in range(NDEV):
        b, i = divmod(dev, DPB)
        sl = slice(SQ * i, SQ * (i + 1))
        m = {
            "xT": np.ascontiguousarray(x[b, sl, :].T.astype(np.float16)),
            "wq": wq_h,
            "wk": wk_h,
            "wv": wv_h,
            "wo": wo_h,
            "cosT": np.ascontiguousarray(cos[b, sl, :].T),
            "sinT": np.ascontiguousarray(sin[b, sl, :].T),
        }
        if with_bias_qkv:
            m["bqd"] = np.ascontiguousarray(np.asarray(bq, np.float32).reshape(HQ, D).T)
            m["bkd"] = np.ascontiguousarray(np.asarray(bk, np.float32).reshape(HKV, D).T)
            m["bvd"] = np.ascontiguousarray(np.asarray(bv, np.float32).reshape(HKV, D).T)
        if with_bias_o:
            m["bod"] = np.asarray(bo, np.float32).reshape(1, E)
        in_maps.append(m)

    res = run_bass_kernel_spmd(nc, in_maps, list(range(NDEV)), **RUN_KWARGS)
    kernel.last_result = res

    out = np.empty((B, S, E), dtype=np.float32)
    for dev in range(NDEV):
        b, i = divmod(dev, DPB)
        out[b, SQ * i : SQ * (i + 1), :] = res.results[dev]["out"]
    return out

